# revision 1
# baseline (speedup 1.0000x reference)
"""Trainium2 Bass kernel for nn_IdentityConvolution.

reference semantics:
    r = sum_c x_real[b, c, :, :]   # [B, 1, H, W]
    i = sum_c x_imag[b, c, :, :]
    out = complex(r, i) broadcast to [B, 64, H, W]  (complex64)

Sharding: data-parallel over batch B=8 across the 8 NeuronCores (one
batch image per core, no cross-core communication).

Per-core device program (fully unrolled, Tile-scheduled):
  - inputs  x_real/x_imag viewed as [C=64, P=128, Q=512] (hw = p*512+q)
  - for each q-chunk: load [128, 16, qc] tiles (full 128 partitions,
    >=1KB contiguous per descriptor), tree-add 16 channels per group on
    the DVE, accumulate 4 groups into acc_r/acc_i [128, qc]
  - interleave acc_r/acc_i into an SBUF tile [128, 2*qc] matching the
    complex64 byte layout
  - DMA that tile to all 64 output-channel planes (contiguous blocks)
"""

import sys

sys.path.insert(0, "/opt/trn_rl_repo")

from contextlib import ExitStack

import numpy as np

import concourse.bass as bass
import concourse.bacc as bacc
import concourse.tile as tile
from concourse import mybir
from concourse.bass_utils import run_bass_kernel_spmd

B, C, H, W = 8, 64, 256, 256
P = 128
Q = (H * W) // P  # 512
NCG = 4  # channel groups
CG = C // NCG  # 16 channels per group
NHW = 2  # q chunks
QC = Q // NHW  # 256

F32 = mybir.dt.float32

_cache = {}


def _build_program(
    repeat=1,
    barrier=False,
    nhw=NHW,
    ncg=NCG,
    split_out=False,
    split_in=False,
    bcast=0,
    inbufs=4,
    dup=1,
):
    qc = Q // nhw
    cg = C // ncg
    nc = bacc.Bacc("TRN2", target_bir_lowering=False, debug=False, num_devices=8)
    xr = nc.dram_tensor("x_real", [C, P, Q], F32, kind="ExternalInput").ap()
    xi = nc.dram_tensor("x_imag", [C, P, Q], F32, kind="ExternalInput").ap()
    out = nc.dram_tensor("out", [C, P, 2 * Q], F32, kind="ExternalOutput").ap()

    xr_v = xr.rearrange("c p q -> p c q")
    xi_v = xi.rearrange("c p q -> p c q")

    with tile.TileContext(nc) as tc, ExitStack() as ctx:
        inp = ctx.enter_context(tc.tile_pool(name="inp", bufs=inbufs))
        scr = ctx.enter_context(tc.tile_pool(name="scr", bufs=2))
        accp = ctx.enter_context(tc.tile_pool(name="acc", bufs=2))
        outp = ctx.enter_context(tc.tile_pool(name="outp", bufs=2))

        for j in range(nhw * repeat):
            if barrier and j and j % nhw == 0:
                tc.strict_bb_all_engine_barrier()
            j = j % nhw
            q0 = j * qc
            acc_r = accp.tile([P, qc], F32, tag="acc_r")
            acc_i = accp.tile([P, qc], F32, tag="acc_i")
            for x_v, acc in ((xr_v, acc_r), (xi_v, acc_i)):
                for g in range(ncg):
                    t = inp.tile([P, cg, qc], F32, tag="in")
                    ieng = nc.scalar if (split_in and g % 2) else nc.sync
                    ieng.dma_start(
                        out=t[:],
                        in_=x_v[:, g * cg : (g + 1) * cg, q0 : q0 + qc],
                    )
                    # first tree level reads the big tile exactly once so
                    # the slot's next DMA writer has few sync waits
                    s = scr.tile([P, cg // 2, qc], F32, tag="s")
                    h = cg // 2
                    nc.vector.tensor_add(s[:], t[:, 0:h, :], t[:, h:cg, :])
                    m = h
                    while m > 1:
                        m //= 2
                        nc.vector.tensor_add(
                            s[:, 0:m, :], s[:, 0:m, :], s[:, m : 2 * m, :]
                        )
                    top = s[:, 0, :]
                    if g == 0:
                        nc.vector.tensor_copy(acc[:], top)
                    else:
                        nc.vector.tensor_add(acc[:], acc[:], top)

            ot = outp.tile([P, dup * 2 * qc], F32, tag="ot")
            otv = ot[:].rearrange("p (d q t) -> p d q t", d=dup, t=2)
            for d in range(dup):
                nc.vector.tensor_copy(otv[:, d, :, 0], acc_r[:])
                nc.vector.tensor_copy(otv[:, d, :, 1], acc_i[:])
            if bcast:
                src1 = ot[:].unsqueeze(0)
                for co in range(0, C, bcast):
                    eng = nc.scalar if (split_out and (co // bcast) % 2) else nc.sync
                    eng.dma_start(
                        out=out[co : co + bcast, :, 2 * q0 : 2 * q0 + 2 * qc],
                        in_=src1.broadcast_to((bcast, P, 2 * qc)),
                    )
            else:
                for co in range(0, C, dup):
                    eng = nc.scalar if (split_out and (co // dup) % 2) else nc.sync
                    if dup == 1:
                        eng.dma_start(
                            out=out[co, :, 2 * q0 : 2 * q0 + 2 * qc], in_=ot[:]
                        )
                    else:
                        eng.dma_start(
                            out=out[co : co + dup, :, 2 * q0 : 2 * q0 + 2 * qc],
                            in_=ot[:].rearrange("p (d f) -> d p f", d=dup),
                        )
    nc.compile()
    return nc


def kernel(x_real, x_imag, _profile=False):
    if "nc" not in _cache:
        _cache["nc"] = _build_program(split_out=True, split_in=True, inbufs=6)
    nc = _cache["nc"]

    x_real = np.asarray(x_real)
    x_imag = np.asarray(x_imag)
    in_maps = [
        {
            "x_real": np.ascontiguousarray(x_real[b]).reshape(C, P, Q),
            "x_imag": np.ascontiguousarray(x_imag[b]).reshape(C, P, Q),
        }
        for b in range(B)
    ]
    res = run_bass_kernel_spmd(nc, in_maps, list(range(B)), trace=_profile)
    _cache["last_result"] = res

    out = np.empty((B, C, H, W), dtype=np.complex64)
    for b in range(B):
        o = res.results[b]["out"]  # [C, P, 2Q] f32
        out[b] = o.reshape(C, P * Q, 2).view(np.complex64).reshape(C, H, W)
    return out



# revision 2
# speedup vs baseline: 4.3192x; 4.3192x over previous
"""Trainium2 Bass kernel for nn_IdentityConvolution.

reference semantics:
    r = sum_c x_real[b, c, :, :]   # [B, 1, H, W]
    i = sum_c x_imag[b, c, :, :]
    out = complex(r, i) broadcast to [B, 64, H, W]  (complex64)

Sharding: data-parallel over batch B=8 across the 8 NeuronCores (one
batch image per core, no cross-core communication).

Per-core device program (Tile-scheduled), built to minimize the busiest
compute engine (the harness-metric bottleneck) by splitting the channel
reduction across the Pool (GpSimd) and DVE engines in fp16:

  - inputs viewed as [C=64, P=128, Q=512] (hw = p*512 + q), processed in
    nhw=2 q-chunks of 256.
  - per chunk and lane (real/imag): 4 input tiles [128, 16, qc] f32 are
    DMA'd on the SP/Activation HWDGE queues; a level-1 add folds each
    tile's 16 channels to 8 in one op writing fp16 into a shared wide
    scratch [128, 32, qc] (Pool does 7 of the 8 level-1 adds per chunk,
    DVE does 1 — balances engine busy ~23us each).
  - DVE then runs one merged reduction chain 32->16->8->4->2 in fp16
    (2 elem/cycle/lane mode) and the final add writes the complex-
    interleaved f32 output tile directly ([128, q, 2] strided view).
  - the [128, 2*qc] f32 tile is broadcast-DMA'd to all 64 output channel
    planes (8 dma_starts of 8 planes each, stride-0 source AP) on the
    SP/Act queues.

fp16 intermediate precision: inputs are ~N(0,1), channel sums |.| < ~40;
tree rounding gives rel err ~6e-4 vs the 2e-2 gate.
"""

import sys

sys.path.insert(0, "/opt/trn_rl_repo")

from contextlib import ExitStack

import numpy as np

import concourse.bacc as bacc
import concourse.tile as tile
from concourse import mybir
from concourse.bass_utils import run_bass_kernel_spmd

B, C, H, W = 8, 64, 256, 256
P = 128
Q = (H * W) // P  # 512
NHW = 2  # q chunks
QC = Q // NHW  # 256

F32 = mybir.dt.float32
F16 = mybir.dt.float16

_cache = {}


def _build_program(
    repeat=1,
    barrier=False,
    nhw=NHW,
    dve_l1=1,  # level-1 adds per chunk on DVE (of 8); rest on Pool
    out_bcast=8,
    in_q="sasasasa",
    out_q="asasasas",
    inbufs=8,
    scrbufs=2,
):
    qc = Q // nhw
    nc = bacc.Bacc("TRN2", target_bir_lowering=False, debug=False, num_devices=8)
    xr = nc.dram_tensor("x_real", [C, P, Q], F32, kind="ExternalInput").ap()
    xi = nc.dram_tensor("x_imag", [C, P, Q], F32, kind="ExternalInput").ap()
    out = nc.dram_tensor("out", [C, P, 2 * Q], F32, kind="ExternalOutput").ap()
    xr_v = xr.rearrange("c p q -> p c q")
    xi_v = xi.rearrange("c p q -> p c q")
    emap = {"s": nc.sync, "a": nc.scalar, "g": nc.gpsimd, "v": nc.vector}

    with tile.TileContext(nc) as tc, ExitStack() as ctx, nc.allow_low_precision(
        "channel-sum of ~N(0,1) fits fp16; harness tolerance 2e-2"
    ):
        inp = ctx.enter_context(tc.tile_pool(name="inp", bufs=inbufs))
        scr = ctx.enter_context(tc.tile_pool(name="scr", bufs=scrbufs))
        outp = ctx.enter_context(tc.tile_pool(name="outp", bufs=2))
        for r in range(repeat):
            if r and barrier:
                tc.strict_bb_all_engine_barrier()
            for j in range(nhw):
                q0 = j * qc
                ot = outp.tile([P, 2 * qc], F32, tag="ot")
                otv = ot[:].rearrange("p (q t) -> p q t", t=2)
                k = 0
                for lane, x_v in enumerate((xr_v, xi_v)):
                    s = scr.tile([P, 32, qc], F16, tag=f"s{lane}")
                    for g in range(4):
                        t = inp.tile([P, 16, qc], F32, tag="in")
                        emap[in_q[(lane * 4 + g) % len(in_q)]].dma_start(
                            out=t[:],
                            in_=x_v[:, g * 16 : (g + 1) * 16, q0 : q0 + qc],
                        )
                        l1 = nc.vector if k < dve_l1 else nc.gpsimd
                        l1.tensor_add(
                            s[:, g * 8 : g * 8 + 8, :],
                            t[:, 0:8, :],
                            t[:, 8:16, :],
                        )
                        k += 1
                    nc.vector.tensor_add(
                        s[:, 0:16, :], s[:, 0:16, :], s[:, 16:32, :]
                    )
                    nc.vector.tensor_add(s[:, 0:8, :], s[:, 0:8, :], s[:, 8:16, :])
                    nc.vector.tensor_add(s[:, 0:4, :], s[:, 0:4, :], s[:, 4:8, :])
                    nc.vector.tensor_add(s[:, 0:2, :], s[:, 0:2, :], s[:, 2:4, :])
                    nc.vector.tensor_add(otv[:, :, lane], s[:, 0, :], s[:, 1, :])
                for m, co in enumerate(range(0, C, out_bcast)):
                    eng = emap[out_q[m % len(out_q)]]
                    eng.dma_start(
                        out=out[co : co + out_bcast, :, 2 * q0 : 2 * q0 + 2 * qc]
                        .rearrange("c p q -> p c q"),
                        in_=ot[:].unsqueeze(1).broadcast_to((P, out_bcast, 2 * qc)),
                    )
    nc.compile()
    return nc


def kernel(x_real, x_imag, _profile=False):
    if "nc" not in _cache:
        _cache["nc"] = _build_program()
    nc = _cache["nc"]

    x_real = np.asarray(x_real)
    x_imag = np.asarray(x_imag)
    in_maps = [
        {
            "x_real": np.ascontiguousarray(x_real[b]).reshape(C, P, Q),
            "x_imag": np.ascontiguousarray(x_imag[b]).reshape(C, P, Q),
        }
        for b in range(B)
    ]
    res = run_bass_kernel_spmd(nc, in_maps, list(range(B)), trace=_profile)
    _cache["last_result"] = res

    out = np.empty((B, C, H, W), dtype=np.complex64)
    for b in range(B):
        o = res.results[b]["out"]  # [C, P, 2Q] f32
        out[b] = o.reshape(C, P * Q, 2).view(np.complex64).reshape(C, H, W)
    return out


# revision 4
# speedup vs baseline: 4.3387x; 1.0045x over previous
"""Trainium2 Bass kernel for nn_IdentityConvolution.

reference semantics:
    r = sum_c x_real[b, c, :, :]   # [B, 1, H, W]
    i = sum_c x_imag[b, c, :, :]
    out = complex(r, i) broadcast to [B, 64, H, W]  (complex64)

Sharding: data-parallel over batch B=8 across the 8 NeuronCores (one
batch image per core, no cross-core communication).

Per-core device program (Tile-scheduled), built to minimize the busiest
compute engine (the harness-metric bottleneck) by splitting the channel
reduction across the Pool (GpSimd) and DVE engines in fp16:

  - inputs viewed as [C=64, P=128, Q=512] (hw = p*512 + q), processed in
    nhw=2 q-chunks of 256.
  - per chunk and lane (real/imag): 4 input tiles [128, 16, qc] f32 are
    DMA'd on the SP/Activation HWDGE queues; a level-1 add folds each
    tile's 16 channels to 8 in one op writing fp16 into a shared wide
    scratch [128, 32, qc] (Pool does 7 of the 8 level-1 adds per chunk,
    DVE does 1 — balances engine busy ~23us each).
  - DVE then runs one merged reduction chain 32->16->8->4->2 in fp16
    (2 elem/cycle/lane mode) and the final add writes the complex-
    interleaved f32 output tile directly ([128, q, 2] strided view).
  - the [128, 2*qc] f32 tile is broadcast-DMA'd to all 64 output channel
    planes (8 dma_starts of 8 planes each, stride-0 source AP) on the
    SP/Act queues.

fp16 intermediate precision: inputs are ~N(0,1), channel sums |.| < ~40;
tree rounding gives rel err ~6e-4 vs the 2e-2 gate.
"""

import sys

sys.path.insert(0, "/opt/trn_rl_repo")

from contextlib import ExitStack

import numpy as np

import concourse.bacc as bacc
import concourse.tile as tile
from concourse import mybir
from concourse.bass_utils import run_bass_kernel_spmd

B, C, H, W = 8, 64, 256, 256
P = 128
Q = (H * W) // P  # 512
NHW = 2  # q chunks
QC = Q // NHW  # 256

F32 = mybir.dt.float32
F16 = mybir.dt.float16

_cache = {}


def _build_program(
    repeat=1,
    barrier=False,
    nhw=NHW,
    dve_l1=1,  # level-1 adds per chunk on DVE (of 8); rest on Pool
    split_l1=1,  # additionally split this many L1 adds per rep half/half
    out_bcast=8,
    in_q="sasasasa",
    out_q="asasasas",
    inbufs=8,
    scrbufs=2,
):
    qc = Q // nhw
    nc = bacc.Bacc("TRN2", target_bir_lowering=False, debug=False, num_devices=8)
    xr = nc.dram_tensor("x_real", [C, P, Q], F32, kind="ExternalInput").ap()
    xi = nc.dram_tensor("x_imag", [C, P, Q], F32, kind="ExternalInput").ap()
    out = nc.dram_tensor("out", [C, P, 2 * Q], F32, kind="ExternalOutput").ap()
    xr_v = xr.rearrange("c p q -> p c q")
    xi_v = xi.rearrange("c p q -> p c q")
    emap = {"s": nc.sync, "a": nc.scalar, "g": nc.gpsimd, "v": nc.vector}

    with tile.TileContext(nc) as tc, ExitStack() as ctx, nc.allow_low_precision(
        "channel-sum of ~N(0,1) fits fp16; harness tolerance 2e-2"
    ):
        inp = ctx.enter_context(tc.tile_pool(name="inp", bufs=inbufs))
        scr = ctx.enter_context(tc.tile_pool(name="scr", bufs=scrbufs))
        outp = ctx.enter_context(tc.tile_pool(name="outp", bufs=2))
        for r in range(repeat):
            if r and barrier:
                tc.strict_bb_all_engine_barrier()
            for j in range(nhw):
                q0 = j * qc
                ot = outp.tile([P, 2 * qc], F32, tag="ot")
                otv = ot[:].rearrange("p (q t) -> p q t", t=2)
                k = 0
                for lane, x_v in enumerate((xr_v, xi_v)):
                    s = scr.tile([P, 32, qc], F16, tag=f"s{lane}")
                    for g in range(4):
                        t = inp.tile([P, 16, qc], F32, tag="in")
                        emap[in_q[(lane * 4 + g) % len(in_q)]].dma_start(
                            out=t[:],
                            in_=x_v[:, g * 16 : (g + 1) * 16, q0 : q0 + qc],
                        )
                        if k == dve_l1 and j == 0 and split_l1:
                            # fine-grain balance: half this L1 on each engine
                            nc.vector.tensor_add(
                                s[:, g * 8 : g * 8 + 4, :],
                                t[:, 0:4, :],
                                t[:, 8:12, :],
                            )
                            nc.gpsimd.tensor_add(
                                s[:, g * 8 + 4 : g * 8 + 8, :],
                                t[:, 4:8, :],
                                t[:, 12:16, :],
                            )
                        else:
                            l1 = nc.vector if k < dve_l1 else nc.gpsimd
                            l1.tensor_add(
                                s[:, g * 8 : g * 8 + 8, :],
                                t[:, 0:8, :],
                                t[:, 8:16, :],
                            )
                        k += 1
                    nc.vector.tensor_add(
                        s[:, 0:16, :], s[:, 0:16, :], s[:, 16:32, :]
                    )
                    nc.vector.tensor_add(s[:, 0:8, :], s[:, 0:8, :], s[:, 8:16, :])
                    nc.vector.tensor_add(s[:, 0:4, :], s[:, 0:4, :], s[:, 4:8, :])
                    nc.vector.tensor_add(s[:, 0:2, :], s[:, 0:2, :], s[:, 2:4, :])
                    nc.vector.tensor_add(otv[:, :, lane], s[:, 0, :], s[:, 1, :])
                for m, co in enumerate(range(0, C, out_bcast)):
                    eng = emap[out_q[m % len(out_q)]]
                    eng.dma_start(
                        out=out[co : co + out_bcast, :, 2 * q0 : 2 * q0 + 2 * qc]
                        .rearrange("c p q -> p c q"),
                        in_=ot[:].unsqueeze(1).broadcast_to((P, out_bcast, 2 * qc)),
                    )
    nc.compile()
    return nc


def kernel(x_real, x_imag, _profile=False):
    if "nc" not in _cache:
        _cache["nc"] = _build_program()
    nc = _cache["nc"]

    x_real = np.asarray(x_real)
    x_imag = np.asarray(x_imag)
    in_maps = [
        {
            "x_real": np.ascontiguousarray(x_real[b]).reshape(C, P, Q),
            "x_imag": np.ascontiguousarray(x_imag[b]).reshape(C, P, Q),
        }
        for b in range(B)
    ]
    res = run_bass_kernel_spmd(nc, in_maps, list(range(B)), trace=_profile)
    _cache["last_result"] = res

    out = np.empty((B, C, H, W), dtype=np.complex64)
    for b in range(B):
        o = res.results[b]["out"]  # [C, P, 2Q] f32
        out[b] = o.reshape(C, P * Q, 2).view(np.complex64).reshape(C, H, W)
    return out


# revision 5
# speedup vs baseline: 4.3879x; 1.0114x over previous
"""Trainium2 Bass kernel for nn_IdentityConvolution.

reference semantics:
    r = sum_c x_real[b, c, :, :]   # [B, 1, H, W]
    i = sum_c x_imag[b, c, :, :]
    out = complex(r, i) broadcast to [B, 64, H, W]  (complex64)

Sharding: data-parallel over batch B=8 across the 8 NeuronCores (one
batch image per core, no cross-core communication).

Per-core device program (Tile-scheduled), built to minimize the busiest
compute engine (the harness-metric bottleneck) by splitting the channel
reduction across the Pool (GpSimd) and DVE engines in fp16:

  - inputs viewed as [C=64, P=128, Q=512] (hw = p*512 + q), processed in
    nhw=2 q-chunks of 256.
  - per chunk and lane (real/imag): 4 input tiles [128, 16, qc] f32 are
    DMA'd on the SP/Activation HWDGE queues; a level-1 add folds each
    tile's 16 channels to 8 in one op writing fp16 into a shared wide
    scratch [128, 32, qc] (Pool does 7 of the 8 level-1 adds per chunk,
    DVE does 1 — balances engine busy ~23us each).
  - DVE then runs one merged reduction chain 32->16->8->4->2 in fp16
    (2 elem/cycle/lane mode) and the final add writes the complex-
    interleaved f32 output tile directly ([128, q, 2] strided view).
  - the [128, 2*qc] f32 tile is broadcast-DMA'd to all 64 output channel
    planes (8 dma_starts of 8 planes each, stride-0 source AP) on the
    SP/Act queues.

fp16 intermediate precision: inputs are ~N(0,1), channel sums |.| < ~40;
tree rounding gives rel err ~6e-4 vs the 2e-2 gate.
"""

import sys

sys.path.insert(0, "/opt/trn_rl_repo")

from contextlib import ExitStack

import numpy as np

import concourse.bacc as bacc
import concourse.tile as tile
from concourse import mybir
from concourse.bass_utils import run_bass_kernel_spmd

B, C, H, W = 8, 64, 256, 256
P = 128
Q = (H * W) // P  # 512
NHW = 2  # q chunks
QC = Q // NHW  # 256

F32 = mybir.dt.float32
F16 = mybir.dt.float16

_cache = {}


def _build_program(
    repeat=1,
    barrier=False,
    nhw=NHW,
    dve_l1=1,  # level-1 adds per chunk on DVE (of 8); rest on Pool
    split_l1=1,  # additionally split this many L1 adds per rep half/half
    out_bcast=8,
    in_q="sasasasa",
    out_q="asasasas",
    inbufs=8,
    scrbufs=2,
):
    qc = Q // nhw
    nc = bacc.Bacc("TRN2", target_bir_lowering=False, debug=False, num_devices=8)
    xr = nc.dram_tensor("x_real", [C, P, Q], F32, kind="ExternalInput").ap()
    xi = nc.dram_tensor("x_imag", [C, P, Q], F32, kind="ExternalInput").ap()
    out = nc.dram_tensor("out", [C, P, 2 * Q], F32, kind="ExternalOutput").ap()
    xr_v = xr.rearrange("c p q -> p c q")
    xi_v = xi.rearrange("c p q -> p c q")
    emap = {"s": nc.sync, "a": nc.scalar, "g": nc.gpsimd, "v": nc.vector}

    with tile.TileContext(nc) as tc, ExitStack() as ctx, nc.allow_low_precision(
        "channel-sum of ~N(0,1) fits fp16; harness tolerance 2e-2"
    ):
        inp = ctx.enter_context(tc.tile_pool(name="inp", bufs=inbufs))
        scr = ctx.enter_context(tc.tile_pool(name="scr", bufs=scrbufs))
        outp = ctx.enter_context(tc.tile_pool(name="outp", bufs=2))
        for r in range(repeat):
            if r and barrier:
                tc.strict_bb_all_engine_barrier()
            for j in range(nhw):
                q0 = j * qc
                ot = outp.tile([P, 2 * qc], F32, tag="ot")
                otv = ot[:].rearrange("p (q t) -> p q t", t=2)
                k = 0
                for lane, x_v in enumerate((xr_v, xi_v)):
                    s = scr.tile([P, 32, qc], F16, tag=f"s{lane}")
                    for g in range(4):
                        t = inp.tile([P, 16, qc], F32, tag="in")
                        emap[in_q[(lane * 4 + g) % len(in_q)]].dma_start(
                            out=t[:],
                            in_=x_v[:, g * 16 : (g + 1) * 16, q0 : q0 + qc],
                        )
                        if k == dve_l1 and j == 0 and split_l1:
                            # fine-grain balance: split this L1 3/5 DVE/Pool
                            h = split_l1 if split_l1 > 1 else 3
                            nc.vector.tensor_add(
                                s[:, g * 8 : g * 8 + h, :],
                                t[:, 0:h, :],
                                t[:, 8 : 8 + h, :],
                            )
                            nc.gpsimd.tensor_add(
                                s[:, g * 8 + h : g * 8 + 8, :],
                                t[:, h:8, :],
                                t[:, 8 + h : 16, :],
                            )
                        else:
                            l1 = nc.vector if k < dve_l1 else nc.gpsimd
                            l1.tensor_add(
                                s[:, g * 8 : g * 8 + 8, :],
                                t[:, 0:8, :],
                                t[:, 8:16, :],
                            )
                        k += 1
                    nc.vector.tensor_add(
                        s[:, 0:16, :], s[:, 0:16, :], s[:, 16:32, :]
                    )
                    nc.vector.tensor_add(s[:, 0:8, :], s[:, 0:8, :], s[:, 8:16, :])
                    nc.vector.tensor_add(s[:, 0:4, :], s[:, 0:4, :], s[:, 4:8, :])
                    nc.vector.tensor_add(s[:, 0:2, :], s[:, 0:2, :], s[:, 2:4, :])
                    nc.vector.tensor_add(otv[:, :, lane], s[:, 0, :], s[:, 1, :])
                for m, co in enumerate(range(0, C, out_bcast)):
                    eng = emap[out_q[m % len(out_q)]]
                    eng.dma_start(
                        out=out[co : co + out_bcast, :, 2 * q0 : 2 * q0 + 2 * qc]
                        .rearrange("c p q -> p c q"),
                        in_=ot[:].unsqueeze(1).broadcast_to((P, out_bcast, 2 * qc)),
                    )
    nc.compile()
    return nc


def kernel(x_real, x_imag, _profile=False):
    if "nc" not in _cache:
        _cache["nc"] = _build_program()
    nc = _cache["nc"]

    x_real = np.asarray(x_real)
    x_imag = np.asarray(x_imag)
    in_maps = [
        {
            "x_real": np.ascontiguousarray(x_real[b]).reshape(C, P, Q),
            "x_imag": np.ascontiguousarray(x_imag[b]).reshape(C, P, Q),
        }
        for b in range(B)
    ]
    res = run_bass_kernel_spmd(nc, in_maps, list(range(B)), trace=_profile)
    _cache["last_result"] = res

    out = np.empty((B, C, H, W), dtype=np.complex64)
    for b in range(B):
        o = res.results[b]["out"]  # [C, P, 2Q] f32
        out[b] = o.reshape(C, P * Q, 2).view(np.complex64).reshape(C, H, W)
    return out


# revision 7
# speedup vs baseline: 4.3978x; 1.0023x over previous
"""Trainium2 Bass kernel for nn_IdentityConvolution.

reference semantics:
    r = sum_c x_real[b, c, :, :]   # [B, 1, H, W]
    i = sum_c x_imag[b, c, :, :]
    out = complex(r, i) broadcast to [B, 64, H, W]  (complex64)

Sharding: data-parallel over batch B=8 across the 8 NeuronCores (one
batch image per core, no cross-core communication).

Per-core device program (Tile-scheduled), built to minimize the busiest
compute engine (the harness-metric bottleneck) by splitting the channel
reduction across the Pool (GpSimd) and DVE engines in fp16:

  - inputs viewed as [C=64, P=128, Q=512] (hw = p*512 + q), processed in
    nhw=2 q-chunks of 256.
  - per chunk and lane (real/imag): 4 input tiles [128, 16, qc] f32 are
    DMA'd on the SP/Activation HWDGE queues; a level-1 add folds each
    tile's 16 channels to 8 in one op writing fp16 into a shared wide
    scratch [128, 32, qc] (Pool does 7 of the 8 level-1 adds per chunk,
    DVE does 1 — balances engine busy ~23us each).
  - DVE then runs one merged reduction chain 32->16->8->4->2 in fp16
    (2 elem/cycle/lane mode) and the final add writes the complex-
    interleaved f32 output tile directly ([128, q, 2] strided view).
  - the [128, 2*qc] f32 tile is broadcast-DMA'd to all 64 output channel
    planes (8 dma_starts of 8 planes each, stride-0 source AP) on the
    SP/Act queues.

fp16 intermediate precision: inputs are ~N(0,1), channel sums |.| < ~40;
tree rounding gives rel err ~6e-4 vs the 2e-2 gate.
"""

import sys

sys.path.insert(0, "/opt/trn_rl_repo")

from contextlib import ExitStack

import numpy as np

import concourse.bacc as bacc
import concourse.tile as tile
from concourse import mybir
from concourse.bass_utils import run_bass_kernel_spmd

B, C, H, W = 8, 64, 256, 256
P = 128
Q = (H * W) // P  # 512
NHW = 2  # q chunks
QC = Q // NHW  # 256

F32 = mybir.dt.float32
F16 = mybir.dt.float16

_cache = {}


def _build_program(
    repeat=1,
    barrier=False,
    nhw=NHW,
    dve_l1=1,  # level-1 adds per chunk on DVE (of 8); rest on Pool
    split_l1=2,  # split one L1 add per rep: this many channel-pairs on DVE
    out_bcast=8,
    in_q="sasasasa",
    out_q="asasasas",
    inbufs=8,
    scrbufs=2,
):
    qc = Q // nhw
    nc = bacc.Bacc("TRN2", target_bir_lowering=False, debug=False, num_devices=8)
    xr = nc.dram_tensor("x_real", [C, P, Q], F32, kind="ExternalInput").ap()
    xi = nc.dram_tensor("x_imag", [C, P, Q], F32, kind="ExternalInput").ap()
    out = nc.dram_tensor("out", [C, P, 2 * Q], F32, kind="ExternalOutput").ap()
    xr_v = xr.rearrange("c p q -> p c q")
    xi_v = xi.rearrange("c p q -> p c q")
    emap = {"s": nc.sync, "a": nc.scalar, "g": nc.gpsimd, "v": nc.vector}

    with tile.TileContext(nc) as tc, ExitStack() as ctx, nc.allow_low_precision(
        "channel-sum of ~N(0,1) fits fp16; harness tolerance 2e-2"
    ):
        inp = ctx.enter_context(tc.tile_pool(name="inp", bufs=inbufs))
        scr = ctx.enter_context(tc.tile_pool(name="scr", bufs=scrbufs))
        outp = ctx.enter_context(tc.tile_pool(name="outp", bufs=2))
        for r in range(repeat):
            if r and barrier:
                tc.strict_bb_all_engine_barrier()
            for j in range(nhw):
                q0 = j * qc
                ot = outp.tile([P, 2 * qc], F32, tag="ot")
                otv = ot[:].rearrange("p (q t) -> p q t", t=2)
                k = 0
                for lane, x_v in enumerate((xr_v, xi_v)):
                    s = scr.tile([P, 32, qc], F16, tag=f"s{lane}")
                    for g in range(4):
                        t = inp.tile([P, 16, qc], F32, tag="in")
                        emap[in_q[(lane * 4 + g) % len(in_q)]].dma_start(
                            out=t[:],
                            in_=x_v[:, g * 16 : (g + 1) * 16, q0 : q0 + qc],
                        )
                        if k == dve_l1 and j == 0 and split_l1:
                            # fine-grain balance: split this L1 h/(8-h)
                            h = split_l1
                            nc.vector.tensor_add(
                                s[:, g * 8 : g * 8 + h, :],
                                t[:, 0:h, :],
                                t[:, 8 : 8 + h, :],
                            )
                            nc.gpsimd.tensor_add(
                                s[:, g * 8 + h : g * 8 + 8, :],
                                t[:, h:8, :],
                                t[:, 8 + h : 16, :],
                            )
                        else:
                            l1 = nc.vector if k < dve_l1 else nc.gpsimd
                            l1.tensor_add(
                                s[:, g * 8 : g * 8 + 8, :],
                                t[:, 0:8, :],
                                t[:, 8:16, :],
                            )
                        k += 1
                    nc.vector.tensor_add(
                        s[:, 0:16, :], s[:, 0:16, :], s[:, 16:32, :]
                    )
                    nc.vector.tensor_add(s[:, 0:8, :], s[:, 0:8, :], s[:, 8:16, :])
                    nc.vector.tensor_add(s[:, 0:4, :], s[:, 0:4, :], s[:, 4:8, :])
                    nc.vector.tensor_add(s[:, 0:2, :], s[:, 0:2, :], s[:, 2:4, :])
                    nc.vector.tensor_add(otv[:, :, lane], s[:, 0, :], s[:, 1, :])
                for m, co in enumerate(range(0, C, out_bcast)):
                    eng = emap[out_q[m % len(out_q)]]
                    eng.dma_start(
                        out=out[co : co + out_bcast, :, 2 * q0 : 2 * q0 + 2 * qc]
                        .rearrange("c p q -> p c q"),
                        in_=ot[:].unsqueeze(1).broadcast_to((P, out_bcast, 2 * qc)),
                    )
    nc.compile()
    return nc


def kernel(x_real, x_imag, _profile=False):
    if "nc" not in _cache:
        _cache["nc"] = _build_program()
    nc = _cache["nc"]

    x_real = np.asarray(x_real)
    x_imag = np.asarray(x_imag)
    in_maps = [
        {
            "x_real": np.ascontiguousarray(x_real[b]).reshape(C, P, Q),
            "x_imag": np.ascontiguousarray(x_imag[b]).reshape(C, P, Q),
        }
        for b in range(B)
    ]
    res = run_bass_kernel_spmd(nc, in_maps, list(range(B)), trace=_profile)
    _cache["last_result"] = res

    out = np.empty((B, C, H, W), dtype=np.complex64)
    for b in range(B):
        o = res.results[b]["out"]  # [C, P, 2Q] f32
        out[b] = o.reshape(C, P * Q, 2).view(np.complex64).reshape(C, H, W)
    return out


# revision 10
# speedup vs baseline: 4.4379x; 1.0091x over previous
"""Trainium2 Bass kernel for nn_IdentityConvolution.

reference semantics:
    r = sum_c x_real[b, c, :, :]   # [B, 1, H, W]
    i = sum_c x_imag[b, c, :, :]
    out = complex(r, i) broadcast to [B, 64, H, W]  (complex64)

Sharding: data-parallel over batch B=8 across the 8 NeuronCores (one
batch image per core, no cross-core communication).

Per-core device program (Tile-scheduled), built to minimize the busiest
compute engine (the harness-metric bottleneck) by splitting the channel
reduction across the Pool (GpSimd) and DVE engines in fp16:

  - inputs viewed as [C=64, P=128, Q=512] (hw = p*512 + q), processed in
    nhw=2 q-chunks of 256.
  - per chunk and lane (real/imag): 4 input tiles [128, 16, qc] f32 are
    DMA'd on the SP/Activation HWDGE queues; a level-1 add folds each
    tile's 16 channels to 8 in one op writing fp16 into a shared wide
    scratch [128, 32, qc] (Pool does 7 of the 8 level-1 adds per chunk,
    DVE does 1 — balances engine busy ~23us each).
  - DVE then runs one merged reduction chain 32->16->8->4->2 in fp16
    (2 elem/cycle/lane mode) and the final add writes the complex-
    interleaved f32 output tile directly ([128, q, 2] strided view).
  - the [128, 2*qc] f32 tile is broadcast-DMA'd to all 64 output channel
    planes (8 dma_starts of 8 planes each, stride-0 source AP) on the
    SP/Act queues.

fp16 intermediate precision: inputs are ~N(0,1), channel sums |.| < ~40;
tree rounding gives rel err ~6e-4 vs the 2e-2 gate.
"""

import sys

sys.path.insert(0, "/opt/trn_rl_repo")

from contextlib import ExitStack

import numpy as np

import concourse.bacc as bacc
import concourse.tile as tile
from concourse import mybir
from concourse.bass_utils import run_bass_kernel_spmd

B, C, H, W = 8, 64, 256, 256
P = 128
Q = (H * W) // P  # 512
NHW = 2  # q chunks
QC = Q // NHW  # 256

F32 = mybir.dt.float32
F16 = mybir.dt.float16

_cache = {}


def _build_program(
    repeat=1,
    barrier=False,
    nhw=NHW,
    dve_l1=1,  # level-1 adds per chunk on DVE (of 8); rest on Pool
    split_l1=4,  # split one L1 add per rep: this many channel-pairs on DVE
    out_bcast=8,
    in_q="sasasasa",
    out_q="asasasas",
    inbufs=8,
    scrbufs=2,
):
    qc = Q // nhw
    nc = bacc.Bacc("TRN2", target_bir_lowering=False, debug=False, num_devices=8)
    xr = nc.dram_tensor("x_real", [C, P, Q], F32, kind="ExternalInput").ap()
    xi = nc.dram_tensor("x_imag", [C, P, Q], F32, kind="ExternalInput").ap()
    out = nc.dram_tensor("out", [C, P, 2 * Q], F32, kind="ExternalOutput").ap()
    xr_v = xr.rearrange("c p q -> p c q")
    xi_v = xi.rearrange("c p q -> p c q")
    emap = {"s": nc.sync, "a": nc.scalar, "g": nc.gpsimd, "v": nc.vector}

    with tile.TileContext(nc) as tc, ExitStack() as ctx, nc.allow_low_precision(
        "channel-sum of ~N(0,1) fits fp16; harness tolerance 2e-2"
    ):
        inp = ctx.enter_context(tc.tile_pool(name="inp", bufs=inbufs))
        scr = ctx.enter_context(tc.tile_pool(name="scr", bufs=scrbufs))
        accp = ctx.enter_context(tc.tile_pool(name="acc", bufs=2))
        outp = ctx.enter_context(tc.tile_pool(name="outp", bufs=2))
        for r in range(repeat):
            if r and barrier:
                tc.strict_bb_all_engine_barrier()
            for j in range(nhw):
                q0 = j * qc
                ot = outp.tile([P, 2 * qc], F32, tag="ot")
                otv = ot[:].rearrange("p (q t) -> p q t", t=2)
                k = 0
                for lane, x_v in enumerate((xr_v, xi_v)):
                    s = scr.tile([P, 32, qc], F16, tag=f"s{lane}")
                    for g in range(4):
                        t = inp.tile([P, 16, qc], F32, tag="in")
                        emap[in_q[(lane * 4 + g) % len(in_q)]].dma_start(
                            out=t[:],
                            in_=x_v[:, g * 16 : (g + 1) * 16, q0 : q0 + qc],
                        )
                        if k == dve_l1 and j == 0 and split_l1:
                            # fine-grain balance: split this L1 h/(8-h)
                            h = split_l1
                            nc.vector.tensor_add(
                                s[:, g * 8 : g * 8 + h, :],
                                t[:, 0:h, :],
                                t[:, 8 : 8 + h, :],
                            )
                            nc.gpsimd.tensor_add(
                                s[:, g * 8 + h : g * 8 + 8, :],
                                t[:, h:8, :],
                                t[:, 8 + h : 16, :],
                            )
                        else:
                            l1 = nc.vector if k < dve_l1 else nc.gpsimd
                            l1.tensor_add(
                                s[:, g * 8 : g * 8 + 8, :],
                                t[:, 0:8, :],
                                t[:, 8:16, :],
                            )
                        k += 1
                    nc.vector.tensor_add(
                        s[:, 0:16, :], s[:, 0:16, :], s[:, 16:32, :]
                    )
                    nc.vector.tensor_add(s[:, 0:8, :], s[:, 0:8, :], s[:, 8:16, :])
                    nc.vector.tensor_add(s[:, 0:4, :], s[:, 0:4, :], s[:, 4:8, :])
                    nc.vector.tensor_add(s[:, 0:2, :], s[:, 0:2, :], s[:, 2:4, :])
                    # final level in fp16 (2x mode), then the idle Act engine
                    # does the cast + complex-interleave write
                    acc = accp.tile([P, qc], F16, tag=f"acc{lane}")
                    nc.vector.tensor_add(acc[:], s[:, 0, :], s[:, 1, :])
                    nc.scalar.activation(
                        otv[:, :, lane],
                        acc[:],
                        mybir.ActivationFunctionType.Copy,
                    )
                for m, co in enumerate(range(0, C, out_bcast)):
                    eng = emap[out_q[m % len(out_q)]]
                    eng.dma_start(
                        out=out[co : co + out_bcast, :, 2 * q0 : 2 * q0 + 2 * qc]
                        .rearrange("c p q -> p c q"),
                        in_=ot[:].unsqueeze(1).broadcast_to((P, out_bcast, 2 * qc)),
                    )
    nc.compile()
    return nc


def kernel(x_real, x_imag, _profile=False):
    if "nc" not in _cache:
        _cache["nc"] = _build_program()
    nc = _cache["nc"]

    x_real = np.asarray(x_real)
    x_imag = np.asarray(x_imag)
    in_maps = [
        {
            "x_real": np.ascontiguousarray(x_real[b]).reshape(C, P, Q),
            "x_imag": np.ascontiguousarray(x_imag[b]).reshape(C, P, Q),
        }
        for b in range(B)
    ]
    res = run_bass_kernel_spmd(nc, in_maps, list(range(B)), trace=_profile)
    _cache["last_result"] = res

    out = np.empty((B, C, H, W), dtype=np.complex64)
    for b in range(B):
        o = res.results[b]["out"]  # [C, P, 2Q] f32
        out[b] = o.reshape(C, P * Q, 2).view(np.complex64).reshape(C, H, W)
    return out
